# revision 10
# baseline (speedup 1.0000x reference)
"""Trainium2 Bass kernel for nn_CustomLayer_22428319220577 (v5).

Math (reference):
    G    = Gmin + (W - Wmin) * a,  a = (Gmax-Gmin)/(Wmax-Wmin)
    G_q  = round((G-Gmin)/(Gmax-Gmin)*15) * (Gmax-Gmin)/15 + Gmin
    Geff = 1/(1/G_q + Rp*((M-i)+(j+1)))
    C    = x @ Geff ;  I = x @ G_q
    coeff= (rowrange I)/(rowrange C + EPS)
    C2   = (C - rowmean C)*coeff + rowmean I
    out  = (C2 - rowsum(x)*b)/a + bias,  b = Gmin - a*Wmin

v5 formulation (same numerics as v3; restructured schedule):
    Gc = G_q - cG ;  H = Geff
    u  = (rowmean_j G_q - b)/a ;  mv = rowmean_j Geff
    A' = x@Gc (cols 1022/1023 replaced by the u/mv product columns)
    B' = x@H
    coeff = rowrange(A'[:, :1022]) / (rowrange(B') + EPS)
    out   = coeff/a * (B' - d2) + d1 + bias      (d1 = x@u, d2 = x@mv)

v5 schedule changes vs v3:
  - x is cast to bf16 (ACT) and transposed by the DMA xbar
    (dma_start_transpose), freeing the PE transposes and the DVE
    PSUM->SBUF copies.  lhsT = bf16, rhs = f32r (both 1 cycle/row).
  - per-sample stats via tensor_scalar+accum_out (max/min reduce at
    4x DVE rate on bf16) instead of tensor_reduce (1x).
  - quantized-G materialization on gpsimd (scalar_tensor_tensor with
    sum-accum providing acc1), freeing one ACT pass per kb.
  - A and B PSUM groups both double-buffered (8 banks, no transpose
    banks needed).
"""
import os
import sys

sys.path.insert(0, "/opt/trn_rl_repo")

from contextlib import ExitStack

import numpy as np

import concourse.bass as bass
import concourse.tile as tile
from concourse import bacc, mybir
from concourse import bass_isa
from concourse.bass_utils import run_bass_kernel_spmd

B_FULL, K, N = 8192, 1024, 1024
N_CORES = 8
B_SH = B_FULL // N_CORES
MT = B_SH // 128
KB = K // 128

R_HRS, R_LRS, RP, BITS, EPS = 40000.0, 1000.0, 2.0, 4, 1e-8
GMIN, GMAX = 1.0 / R_HRS, 1.0 / R_LRS
GSPAN32 = np.float32(GMAX) - np.float32(GMIN)
RSPANG = float(np.float32(1.0) / GSPAN32)
LEVELS = float(2**BITS - 1)
C2_IMM = float(np.float32(GSPAN32) / np.float32(LEVELS))
CG = float(np.float32(0.5) * (np.float32(GMIN) + np.float32(GMAX)))
GMC = float(np.float32(GMIN) - np.float32(CG))

FP32 = mybir.dt.float32
F32R = mybir.dt.float32r
BF16 = mybir.dt.bfloat16
I32 = mybir.dt.int32
MM_DT = BF16


def _build():
    nc = bacc.Bacc("TRN2", target_bir_lowering=False, debug=False,
                   num_devices=N_CORES)

    xs = nc.dram_tensor("xs", [B_SH, K], FP32, kind="ExternalInput").ap()
    w = nc.dram_tensor("w", [K, N], FP32, kind="ExternalInput").ap()
    bias_d = nc.dram_tensor("bias", [N], FP32, kind="ExternalInput").ap()
    offs_d = nc.dram_tensor("offs", [128, KB], FP32, kind="ExternalInput").ap()
    out_d = nc.dram_tensor("out", [B_SH, N], FP32, kind="ExternalOutput").ap()

    AL = mybir.AluOpType
    AX = mybir.AxisListType

    def act_recip(dst, src, bias=0.0, scale=1.0, accum=None):
        # raw ACT Reciprocal: dst = 1/(src*scale + bias)
        eng = nc.scalar
        ins = [eng.lower_ap(src),
               mybir.ImmediateValue(dtype=mybir.dt.float32, value=float(bias)),
               mybir.ImmediateValue(dtype=mybir.dt.float32, value=float(scale)),
               mybir.ImmediateValue(dtype=mybir.dt.float32, value=0.0)]
        outs = [eng.lower_ap(dst)]
        if accum is not None:
            outs.append(eng.lower_ap(accum))
        eng.add_instruction(mybir.InstActivation(
            name=nc.get_next_instruction_name(),
            func=mybir.ActivationFunctionType.Reciprocal,
            ins=ins, outs=outs))

    with tile.TileContext(nc) as tc, ExitStack() as ctx:
        consts = ctx.enter_context(tc.tile_pool(name="consts", bufs=1))
        wk = ctx.enter_context(tc.tile_pool(name="wk", bufs=1))
        zp = ctx.enter_context(tc.tile_pool(name="zp", bufs=1))
        stats = ctx.enter_context(tc.tile_pool(name="stats", bufs=1))
        wtp = ctx.enter_context(tc.tile_pool(name="wtp", bufs=2))
        xin = ctx.enter_context(tc.tile_pool(name="xin", bufs=2))
        xbp = ctx.enter_context(tc.tile_pool(name="xbp", bufs=2))
        xtb_p = ctx.enter_context(tc.tile_pool(name="xtb_p", bufs=2))
        xtk = ctx.enter_context(tc.tile_pool(name="xtk", bufs=1))
        asp = ctx.enter_context(tc.tile_pool(name="asp", bufs=2))
        bsp = ctx.enter_context(tc.tile_pool(name="bsp", bufs=2))
        jnk = ctx.enter_context(tc.tile_pool(name="jnk", bufs=1))
        ob1 = ctx.enter_context(tc.tile_pool(name="ob1", bufs=1))
        obp = ctx.enter_context(tc.tile_pool(name="obp", bufs=2))
        smp = ctx.enter_context(tc.tile_pool(name="smp", bufs=2))
        pap = ctx.enter_context(tc.tile_pool(name="pap", bufs=2, space="PSUM"))
        pbp = ctx.enter_context(tc.tile_pool(name="pbp", bufs=2, space="PSUM"))

        # ---------- phase 0 ----------
        # jif[p, j] = j - p (fp32, exact); RP scaling folded into the den op
        jif = consts.tile([128, N], FP32)
        nc.gpsimd.iota(jif[:], pattern=[[1, N]], base=0, channel_multiplier=-1,
                       allow_small_or_imprecise_dtypes=True)

        biasb = consts.tile([128, N], FP32)
        nc.scalar.dma_start(
            out=biasb[:],
            in_=bass.AP(tensor=bias_d.tensor, offset=bias_d.offset,
                        ap=[[0, 128]] + bias_d.ap),
        )
        offs = consts.tile([128, KB], FP32)
        nc.scalar.dma_start(out=offs[:], in_=offs_d)

        # x row-tiles 0/1 first on the sync ring (so mt0/1 transpose early),
        # then W (stats gate phase 1), then the rest of x.
        xnats = {}
        for mt in (0, 1):
            xnat = xin.tile([128, K], FP32, tag="xn")
            xnats[mt] = xnat
            nc.sync.dma_start(out=xnat[:], in_=xs[mt * 128:(mt + 1) * 128, :])

        # W dma on sync ring; per-kb min/max reduces on DVE
        w8 = wk.tile([128, KB, N], FP32)
        wmax8 = stats.tile([128, KB], FP32)
        wmin8 = stats.tile([128, KB], FP32)
        for kb in range(KB):
            nc.sync.dma_start(out=w8[:, kb, :], in_=w[kb * 128:(kb + 1) * 128, :])
            nc.vector.tensor_reduce(out=wmax8[:, kb:kb + 1], in_=w8[:, kb, :],
                                    axis=AX.X, op=AL.max)
            nc.vector.tensor_reduce(out=wmin8[:, kb:kb + 1], in_=w8[:, kb, :],
                                    axis=AX.X, op=AL.min)

        for mt in range(2, MT):
            xnat = xin.tile([128, K], FP32, tag="xn")
            xnats[mt] = xnat
            nc.sync.dma_start(out=xnat[:], in_=xs[mt * 128:(mt + 1) * 128, :])

        # x cast to bf16 (DVE, with rowsum accumulate for the u-column
        # constant-split) + DMA xbar transpose (scalar ring) into bf16 xt
        xt = xtk.tile([128, MT, KB, 128], MM_DT)
        sumx = stats.tile([128, MT], FP32)
        for mt in range(MT):
            xbf = xbp.tile([128, K], BF16, tag="xb")
            nc.vector.tensor_scalar(out=xbf[:], in0=xnats[mt][:], scalar1=1.0,
                                    scalar2=None, op0=AL.mult, op1=AL.add,
                                    accum_out=sumx[:, mt:mt + 1])
            nc.scalar.dma_start_transpose(out=xt[:, mt, :, :], in_=xbf[:])

        # pack [wmax | -wmin], one partition all-reduce(max)
        pk = stats.tile([128, 2], FP32)
        nc.vector.tensor_reduce(out=pk[:, 0:1], in_=wmax8[:], axis=AX.X, op=AL.max)
        wminp = stats.tile([128, 1], FP32)
        nc.vector.tensor_reduce(out=wminp[:], in_=wmin8[:], axis=AX.X, op=AL.min)
        nc.vector.tensor_scalar(out=pk[:, 1:2], in0=wminp[:], scalar1=-1.0,
                                scalar2=None, op0=AL.mult)
        pkt = stats.tile([128, 2], FP32)
        nc.gpsimd.partition_all_reduce(pkt[:], pk[:], channels=128,
                                       reduce_op=bass_isa.ReduceOp.max)

        # ---------- scalar chain ----------
        wmax_t = pkt[:, 0:1]
        wmin_t = stats.tile([128, 1], FP32)
        nc.vector.tensor_scalar(out=wmin_t[:], in0=pkt[:, 1:2], scalar1=-1.0,
                                scalar2=None, op0=AL.mult)
        span = stats.tile([128, 1], FP32)
        nc.vector.tensor_scalar(out=span[:], in0=wmax_t, scalar1=wmin_t[:],
                                scalar2=None, op0=AL.subtract)
        rspan = stats.tile([128, 1], FP32)
        nc.vector.reciprocal(rspan[:], span[:])
        s15_t = stats.tile([128, 1], FP32)
        nc.vector.tensor_scalar(out=s15_t[:], in0=rspan[:], scalar1=LEVELS,
                                scalar2=None, op0=AL.mult)
        aG_t = stats.tile([128, 1], FP32)
        nc.vector.tensor_scalar(out=aG_t[:], in0=rspan[:], scalar1=float(GSPAN32),
                                scalar2=None, op0=AL.mult)
        inva_t = stats.tile([128, 1], FP32)
        nc.vector.tensor_scalar(out=inva_t[:], in0=span[:], scalar1=RSPANG,
                                scalar2=None, op0=AL.mult)
        b_t = stats.tile([128, 1], FP32)
        nc.vector.tensor_tensor(out=b_t[:], in0=aG_t[:], in1=wmin_t[:],
                                op=AL.mult)
        nc.vector.tensor_scalar(out=b_t[:], in0=b_t[:], scalar1=-1.0,
                                scalar2=GMIN, op0=AL.mult, op1=AL.add)
        gmc_c = stats.tile([128, 1], FP32)
        nc.vector.memset(gmc_c[:], GMC)
        # u-column constant split: uc = (cG - b)/a; b2 = b + uc*a so that
        # u_var = (meanGq - b2)/a = u - uc stays small (bf16-friendly);
        # the uc*sumx part is re-added to d1 in the per-mt post.
        uc_t = stats.tile([128, 1], FP32)
        nc.vector.tensor_scalar(out=uc_t[:], in0=b_t[:], scalar1=CG,
                                scalar2=-1.0, op0=AL.subtract, op1=AL.mult)
        nc.vector.tensor_scalar(out=uc_t[:], in0=uc_t[:], scalar1=inva_t[:],
                                scalar2=None, op0=AL.mult)
        b2_t = stats.tile([128, 1], FP32)
        nc.vector.tensor_scalar(out=b2_t[:], in0=uc_t[:], scalar1=aG_t[:],
                                scalar2=b_t[:], op0=AL.mult, op1=AL.add)

        # ---------- phase 1: per-kb quant pipeline + mt0/1 matmuls ----------
        zsb = zp.tile([128, KB, 2 * N], MM_DT)
        acc1 = stats.tile([128, KB], FP32)
        accQ = stats.tile([128, KB], FP32)

        pa0 = pap.tile([128, N], FP32, tag="pa")
        pa1 = pap.tile([128, N], FP32, tag="pa")
        pb0 = pbp.tile([128, N], FP32, tag="pb")
        pb1 = pbp.tile([128, N], FP32, tag="pb")
        pas = {0: pa0, 1: pa1}
        pbs = {0: pb0, 1: pb1}

        def emit_quant(kb):
            t15 = wtp.tile([128, N], I32, tag="t15")
            nc.vector.tensor_scalar(out=t15[:], in0=w8[:, kb, :],
                                    scalar1=wmin_t[:], scalar2=s15_t[:],
                                    op0=AL.subtract, op1=AL.mult)
            # zsb A-section = Gc = t15*C2 + (GMIN - cG); sum-accum -> acc1
            nc.scalar.activation(out=zsb[:, kb, 0:N], in_=t15[:],
                                 func=mybir.ActivationFunctionType.Identity,
                                 bias=gmc_c[:], scale=C2_IMM,
                                 accum_out=acc1[:, kb:kb + 1])
            inv = wtp.tile([128, N], FP32, tag="inv")
            act_recip(inv[:], t15[:], bias=GMIN, scale=C2_IMM)
            den = wtp.tile([128, N], FP32, tag="den")
            # den = (jif*RP + offs_kb) + inv  (exact Rpar + 1/G_q)
            nc.vector.affine_then_add(den[:], jif[:], inv[:], RP,
                                      offs[:, kb:kb + 1])
            act_recip(zsb[:, kb, N:2 * N], den[:], bias=0.0, scale=1.0,
                      accum=accQ[:, kb:kb + 1])

        def emit_tail(kb):
            # u/mv columns into zsb A-section cols N-2 / N-1
            # acc1 = sum_j Gc -> mean G_q = acc1/N + cG
            macc = smp.tile([128, 1], FP32, tag="macc")
            nc.vector.tensor_scalar(out=macc[:], in0=acc1[:, kb:kb + 1],
                                    scalar1=1.0 / N, scalar2=CG,
                                    op0=AL.mult, op1=AL.add)
            nc.vector.tensor_scalar(out=zsb[:, kb, N - 2:N - 1], in0=macc[:],
                                    scalar1=b2_t[:], scalar2=inva_t[:],
                                    op0=AL.subtract, op1=AL.mult)
            nc.vector.tensor_scalar(out=zsb[:, kb, N - 1:N],
                                    in0=accQ[:, kb:kb + 1],
                                    scalar1=1.0 / N, scalar2=None, op0=AL.mult)

        def emit_A_kb(mt, kb):
            pa = pas[mt]
            lhsT = xt[:, mt, kb, :]
            st, sp = kb == 0, kb == KB - 1
            nc.tensor.matmul(pa[:, 0:512], lhsT, zsb[:, kb, 0:512],
                             start=st, stop=sp)
            nc.tensor.matmul(pa[:, 512:1024], lhsT, zsb[:, kb, 512:1024],
                             start=st, stop=sp)

        def emit_B_kb(mt, kb):
            pb = pbs[mt]
            lhsT = xt[:, mt, kb, :]
            st, sp = kb == 0, kb == KB - 1
            nc.tensor.matmul(pb[:, 0:512], lhsT, zsb[:, kb, N:N + 512],
                             start=st, stop=sp)
            nc.tensor.matmul(pb[:, 512:1024], lhsT, zsb[:, kb, N + 512:2 * N],
                             start=st, stop=sp)

        for kb in range(KB):
            emit_quant(kb)
            if kb >= 1:
                emit_tail(kb - 1)
                emit_A_kb(0, kb - 1)
                emit_A_kb(1, kb - 1)
                emit_B_kb(0, kb - 1)
                emit_B_kb(1, kb - 1)
        emit_tail(KB - 1)
        emit_A_kb(0, KB - 1)
        emit_A_kb(1, KB - 1)
        emit_B_kb(0, KB - 1)
        emit_B_kb(1, KB - 1)

        # ---------- phase 2: drain + remaining matmuls ----------
        junk = jnk.tile([128, N], BF16)

        def emit_A(mt):
            pa = pap.tile([128, N], FP32, tag="pa")
            pas[mt] = pa
            for kb in range(KB):
                emit_A_kb(mt, kb)

        def emit_B(mt):
            pb = pbp.tile([128, N], FP32, tag="pb")
            pbs[mt] = pb
            for kb in range(KB):
                emit_B_kb(mt, kb)

        def stat(src, accum, op):
            nc.vector.tensor_scalar(out=junk[:, 0:src.shape[-1]], in0=src,
                                    scalar1=1.0, scalar2=None, op0=AL.mult,
                                    op1=op, accum_out=accum)

        def emit_post(mt):
            pa, pb = pas[mt], pbs[mt]
            nd2 = smp.tile([128, 1], FP32, tag="nd2")
            nc.vector.tensor_scalar(out=nd2[:], in0=pa[:, 1023:1024],
                                    scalar1=-1.0, scalar2=None, op0=AL.mult)
            d1v = smp.tile([128, 1], FP32, tag="d1v")
            nc.vector.tensor_scalar(out=d1v[:], in0=pa[:, 1022:1023],
                                    scalar1=1.0, scalar2=None, op0=AL.mult)
            d1s = smp.tile([128, 1], FP32, tag="d1s")
            nc.vector.tensor_scalar(out=d1s[:], in0=sumx[:, mt:mt + 1],
                                    scalar1=uc_t[:], scalar2=d1v[:],
                                    op0=AL.mult, op1=AL.add)
            # A stats: bf16 copy (DVE, PSUM src) then 4x-rate accum stats
            asb = asp.tile([128, 1022], BF16, tag="as")
            nc.vector.tensor_scalar(out=asb[:], in0=pa[:, 0:1022], scalar1=1.0,
                                    scalar2=None, op0=AL.mult)
            # centered bf16 copy of B' (ACT, PSUM src; frees pb after final)
            bsc = bsp.tile([128, N], BF16, tag="bs")
            nc.scalar.activation(out=bsc[:], in_=pb[:],
                                 func=mybir.ActivationFunctionType.Identity,
                                 bias=nd2[:], scale=1.0)
            amax = smp.tile([128, 1], FP32, tag="amax")
            amin = smp.tile([128, 1], FP32, tag="amin")
            bmax = smp.tile([128, 1], FP32, tag="bmax")
            bmin = smp.tile([128, 1], FP32, tag="bmin")
            stat(asb[:], amax[:], AL.max)
            stat(asb[:], amin[:], AL.min)
            stat(bsc[:], bmax[:], AL.max)
            stat(bsc[:], bmin[:], AL.min)
            ra = smp.tile([128, 1], FP32, tag="ra")
            nc.vector.tensor_tensor(out=ra[:], in0=amax[:], in1=amin[:],
                                    op=AL.subtract)
            rbe = smp.tile([128, 1], FP32, tag="rbe")
            nc.vector.tensor_scalar(out=rbe[:], in0=bmax[:], scalar1=bmin[:],
                                    scalar2=EPS, op0=AL.subtract, op1=AL.add)
            rc = smp.tile([128, 1], FP32, tag="rc")
            nc.vector.reciprocal(rc[:], rbe[:])
            coeff = smp.tile([128, 1], FP32, tag="coeff")
            nc.vector.tensor_tensor(out=coeff[:], in0=ra[:], in1=rc[:],
                                    op=AL.mult)
            ci = smp.tile([128, 1], FP32, tag="ci")
            nc.vector.tensor_scalar(out=ci[:], in0=coeff[:], scalar1=inva_t[:],
                                    scalar2=None, op0=AL.mult)
            osb = obp.tile([128, N], FP32, tag="osb")
            if mt < MT - 2:
                osb1 = ob1.tile([128, N], FP32, tag="osb1")
                nc.scalar.activation(out=osb1[:], in_=bsc[:],
                                     func=mybir.ActivationFunctionType.Identity,
                                     bias=d1s[:], scale=ci[:])
                nc.gpsimd.tensor_tensor(out=osb[:], in0=osb1[:], in1=biasb[:],
                                        op=AL.add)
            else:
                nc.vector.affine_then_add(osb[:], bsc[:], biasb[:], ci[:],
                                          d1s[:])
            nc.sync.dma_start(out=out_d[mt * 128:(mt + 1) * 128, :], in_=osb[:])

        emit_post(0)
        emit_A(2)
        emit_B(2)
        emit_post(1)
        for mt in range(3, MT):
            emit_A(mt)
            emit_B(mt)
            emit_post(mt - 2)
        emit_post(MT - 2)
        emit_post(MT - 1)

    nc.compile()
    return nc


_NC_CACHE = None


def _get_nc():
    global _NC_CACHE
    if _NC_CACHE is None:
        _NC_CACHE = _build()
    return _NC_CACHE


def _offs_np():
    # offs[p, kb] = Rp*(K + 1 - kb*128), constant over p; the i/j dependence
    # is jif = (j - p) scaled by RP in the den affine op
    kb = np.arange(KB, dtype=np.float64)[None, :]
    p = np.zeros((128, 1), dtype=np.float64)
    return (RP * (K + 1.0 - kb * 128.0) + p).astype(np.float32)


def kernel(x, weight, bias):
    x = np.ascontiguousarray(x, np.float32)
    weight = np.ascontiguousarray(weight, np.float32)
    bias = np.ascontiguousarray(bias, np.float32)
    nc = _get_nc()
    offs = _offs_np()
    in_maps = [
        {"xs": x[c * B_SH:(c + 1) * B_SH], "w": weight, "bias": bias, "offs": offs}
        for c in range(N_CORES)
    ]
    res = run_bass_kernel_spmd(nc, in_maps, core_ids=list(range(N_CORES)))
    return np.concatenate([res.results[c]["out"] for c in range(N_CORES)], axis=0)


# revision 11
# speedup vs baseline: 1.0575x; 1.0575x over previous
"""Trainium2 Bass kernel for nn_CustomLayer_22428319220577 (v6).

Math (reference):
    G    = Gmin + (W - Wmin) * a,  a = (Gmax-Gmin)/(Wmax-Wmin)
    G_q  = round((G-Gmin)/(Gmax-Gmin)*15) * (Gmax-Gmin)/15 + Gmin
    Geff = 1/(1/G_q + Rp*((M-i)+(j+1)))
    C    = x @ Geff ;  I = x @ G_q
    coeff= (rowrange I)/(rowrange C + EPS)
    C2   = (C - rowmean C)*coeff + rowmean I
    out  = (C2 - rowsum(x)*b)/a + bias,  b = Gmin - a*Wmin

v6 formulation:
    Gc = G_q - cG ;  H = Geff
    u  = (rowmean_j G_q - b)/a, split u = uc + u_var (uc = (cG-b)/a)
    A' = x@Gc (cols 1022/1023 replaced by u_var/mv product columns)
    B' = x@H
    coeff = rowrange(A'[:, :1022]) / (rowrange(B') + EPS)
    out   = coeff/a * (B' - d2) + (d1v + uc*rowsum(x)) + bias

Schedule: all input DMA on the sync ring (x0,x1 first, then W, then x2-7);
x cast to bf16 on ACT (rowsum accumulated for the uc term) and transposed
by the DMA xbar on the scalar ring; per-kb quant pipeline (DVE t15/den,
ACT quant/inv/H) with mt0/1 A+B matmuls riding (bf16 lhsT x bf16 rhs);
drain phase with per-sample stats reduced directly from PSUM in fp32 and
the output assembled by ACT (scale/bias from PSUM) + bias add on
gpsimd/DVE.
"""
import os
import sys

sys.path.insert(0, "/opt/trn_rl_repo")

from contextlib import ExitStack

import numpy as np

import concourse.bass as bass
import concourse.tile as tile
from concourse import bacc, mybir
from concourse import bass_isa
from concourse.bass_utils import run_bass_kernel_spmd

B_FULL, K, N = 8192, 1024, 1024
N_CORES = 8
B_SH = B_FULL // N_CORES
MT = B_SH // 128
KB = K // 128

R_HRS, R_LRS, RP, BITS, EPS = 40000.0, 1000.0, 2.0, 4, 1e-8
GMIN, GMAX = 1.0 / R_HRS, 1.0 / R_LRS
GSPAN32 = np.float32(GMAX) - np.float32(GMIN)
RSPANG = float(np.float32(1.0) / GSPAN32)
LEVELS = float(2**BITS - 1)
C2_IMM = float(np.float32(GSPAN32) / np.float32(LEVELS))
CG = float(np.float32(0.5) * (np.float32(GMIN) + np.float32(GMAX)))
GMC = float(np.float32(GMIN) - np.float32(CG))

FP32 = mybir.dt.float32
F32R = mybir.dt.float32r
BF16 = mybir.dt.bfloat16
I32 = mybir.dt.int32
MM_DT = BF16


def _build():
    nc = bacc.Bacc("TRN2", target_bir_lowering=False, debug=False,
                   num_devices=N_CORES)

    xs = nc.dram_tensor("xs", [B_SH, K], FP32, kind="ExternalInput").ap()
    w = nc.dram_tensor("w", [K, N], FP32, kind="ExternalInput").ap()
    bias_d = nc.dram_tensor("bias", [N], FP32, kind="ExternalInput").ap()
    offs_d = nc.dram_tensor("offs", [128, KB], FP32, kind="ExternalInput").ap()
    out_d = nc.dram_tensor("out", [B_SH, N], FP32, kind="ExternalOutput").ap()

    AL = mybir.AluOpType
    AX = mybir.AxisListType

    def act_recip(dst, src, bias=0.0, scale=1.0, accum=None):
        # raw ACT Reciprocal: dst = 1/(src*scale + bias)
        eng = nc.scalar
        ins = [eng.lower_ap(src),
               mybir.ImmediateValue(dtype=mybir.dt.float32, value=float(bias)),
               mybir.ImmediateValue(dtype=mybir.dt.float32, value=float(scale)),
               mybir.ImmediateValue(dtype=mybir.dt.float32, value=0.0)]
        outs = [eng.lower_ap(dst)]
        if accum is not None:
            outs.append(eng.lower_ap(accum))
        eng.add_instruction(mybir.InstActivation(
            name=nc.get_next_instruction_name(),
            func=mybir.ActivationFunctionType.Reciprocal,
            ins=ins, outs=outs))

    with tile.TileContext(nc) as tc, ExitStack() as ctx:
        consts = ctx.enter_context(tc.tile_pool(name="consts", bufs=1))
        wk = ctx.enter_context(tc.tile_pool(name="wk", bufs=1))
        zp = ctx.enter_context(tc.tile_pool(name="zp", bufs=1))
        stats = ctx.enter_context(tc.tile_pool(name="stats", bufs=1))
        wtp = ctx.enter_context(tc.tile_pool(name="wtp", bufs=2))
        xin = ctx.enter_context(tc.tile_pool(name="xin", bufs=3))
        xbp = ctx.enter_context(tc.tile_pool(name="xbp", bufs=2))
        xtk = ctx.enter_context(tc.tile_pool(name="xtk", bufs=1))
        ob1 = ctx.enter_context(tc.tile_pool(name="ob1", bufs=2))
        obp = ctx.enter_context(tc.tile_pool(name="obp", bufs=2))
        smp = ctx.enter_context(tc.tile_pool(name="smp", bufs=2))
        pap = ctx.enter_context(tc.tile_pool(name="pap", bufs=2, space="PSUM"))
        pbp = ctx.enter_context(tc.tile_pool(name="pbp", bufs=2, space="PSUM"))

        # ---------- phase 0: DMAs + W stats ----------
        jif = consts.tile([128, N], FP32)
        nc.gpsimd.iota(jif[:], pattern=[[1, N]], base=0, channel_multiplier=-1,
                       allow_small_or_imprecise_dtypes=True)

        # all input DMA on the sync ring: consts, x0, x1, W0-7, x2-7 —
        # W lands by ~14us (gates phase 1), x0/1 early for the first
        # transposes, the x tail streams behind W.
        biasb = consts.tile([128, N], FP32)
        nc.sync.dma_start(
            out=biasb[:],
            in_=bass.AP(tensor=bias_d.tensor, offset=bias_d.offset,
                        ap=[[0, 128]] + bias_d.ap),
        )
        offs = consts.tile([128, KB], FP32)
        nc.sync.dma_start(out=offs[:], in_=offs_d)

        xnats = {}

        def emit_x_dma(mt):
            xnat = xin.tile([128, K], FP32, tag="xn")
            xnats[mt] = xnat
            nc.sync.dma_start(out=xnat[:], in_=xs[mt * 128:(mt + 1) * 128, :])

        emit_x_dma(0)
        emit_x_dma(1)

        w8 = wk.tile([128, KB, N], FP32)
        wmax8 = stats.tile([128, KB], FP32)
        wmin8 = stats.tile([128, KB], FP32)
        for kb in range(KB):
            nc.sync.dma_start(out=w8[:, kb, :], in_=w[kb * 128:(kb + 1) * 128, :])
            nc.vector.tensor_reduce(out=wmax8[:, kb:kb + 1], in_=w8[:, kb, :],
                                    axis=AX.X, op=AL.max)
            nc.vector.tensor_reduce(out=wmin8[:, kb:kb + 1], in_=w8[:, kb, :],
                                    axis=AX.X, op=AL.min)
        for mt in range(2, MT):
            emit_x_dma(mt)

        # x cast to bf16 on ACT (+ rowsum accum) and xbar transpose on the
        # scalar ring (issue from the ACT queue right after each cast).
        xt = xtk.tile([128, MT, KB, 128], MM_DT)
        sumx = stats.tile([128, MT], FP32)

        def emit_cast_tr(mt):
            xbf = xbp.tile([128, K], BF16, tag="xb")
            nc.scalar.activation(out=xbf[:], in_=xnats[mt][:],
                                 func=mybir.ActivationFunctionType.Identity,
                                 scale=1.0, accum_out=sumx[:, mt:mt + 1])
            nc.scalar.dma_start_transpose(out=xt[:, mt, :, :], in_=xbf[:])

        emit_cast_tr(0)
        emit_cast_tr(1)

        # pack [wmax | -wmin], one partition all-reduce(max)
        pk = stats.tile([128, 2], FP32)
        nc.vector.tensor_reduce(out=pk[:, 0:1], in_=wmax8[:], axis=AX.X, op=AL.max)
        wminp = stats.tile([128, 1], FP32)
        nc.vector.tensor_reduce(out=wminp[:], in_=wmin8[:], axis=AX.X, op=AL.min)
        nc.vector.tensor_scalar(out=pk[:, 1:2], in0=wminp[:], scalar1=-1.0,
                                scalar2=None, op0=AL.mult)
        pkt = stats.tile([128, 2], FP32)
        nc.gpsimd.partition_all_reduce(pkt[:], pk[:], channels=128,
                                       reduce_op=bass_isa.ReduceOp.max)

        # ---------- scalar chain ----------
        wmax_t = pkt[:, 0:1]
        wmin_t = stats.tile([128, 1], FP32)
        nc.vector.tensor_scalar(out=wmin_t[:], in0=pkt[:, 1:2], scalar1=-1.0,
                                scalar2=None, op0=AL.mult)
        span = stats.tile([128, 1], FP32)
        nc.vector.tensor_scalar(out=span[:], in0=wmax_t, scalar1=wmin_t[:],
                                scalar2=None, op0=AL.subtract)
        rspan = stats.tile([128, 1], FP32)
        nc.vector.reciprocal(rspan[:], span[:])
        s15_t = stats.tile([128, 1], FP32)
        nc.vector.tensor_scalar(out=s15_t[:], in0=rspan[:], scalar1=LEVELS,
                                scalar2=None, op0=AL.mult)
        aG_t = stats.tile([128, 1], FP32)
        nc.vector.tensor_scalar(out=aG_t[:], in0=rspan[:], scalar1=float(GSPAN32),
                                scalar2=None, op0=AL.mult)
        inva_t = stats.tile([128, 1], FP32)
        nc.vector.tensor_scalar(out=inva_t[:], in0=span[:], scalar1=RSPANG,
                                scalar2=None, op0=AL.mult)
        b_t = stats.tile([128, 1], FP32)
        nc.vector.tensor_tensor(out=b_t[:], in0=aG_t[:], in1=wmin_t[:],
                                op=AL.mult)
        nc.vector.tensor_scalar(out=b_t[:], in0=b_t[:], scalar1=-1.0,
                                scalar2=GMIN, op0=AL.mult, op1=AL.add)
        gmc_c = stats.tile([128, 1], FP32)
        nc.vector.memset(gmc_c[:], GMC)
        # u-column constant split: uc = (cG - b)/a; b2 = b + uc*a so that
        # u_var = (meanGq - b2)/a = u - uc stays small in bf16; uc*sumx is
        # re-added per-mt in the post phase.
        uc_t = stats.tile([128, 1], FP32)
        nc.vector.tensor_scalar(out=uc_t[:], in0=b_t[:], scalar1=CG,
                                scalar2=-1.0, op0=AL.subtract, op1=AL.mult)
        nc.vector.tensor_scalar(out=uc_t[:], in0=uc_t[:], scalar1=inva_t[:],
                                scalar2=None, op0=AL.mult)
        b2_t = stats.tile([128, 1], FP32)
        nc.vector.tensor_scalar(out=b2_t[:], in0=uc_t[:], scalar1=aG_t[:],
                                scalar2=b_t[:], op0=AL.mult, op1=AL.add)

        # ---------- phase 1: per-kb quant pipeline + mt0/1 matmuls ----------
        zsb = zp.tile([128, KB, 2 * N], MM_DT)
        acc1 = stats.tile([128, KB], FP32)
        accQ = stats.tile([128, KB], FP32)

        pa0 = pap.tile([128, N], FP32, tag="pa")
        pa1 = pap.tile([128, N], FP32, tag="pa")
        pb0 = pbp.tile([128, N], FP32, tag="pb")
        pb1 = pbp.tile([128, N], FP32, tag="pb")
        pas = {0: pa0, 1: pa1}
        pbs = {0: pb0, 1: pb1}

        def emit_quant(kb):
            t15 = wtp.tile([128, N], I32, tag="t15")
            nc.vector.tensor_scalar(out=t15[:], in0=w8[:, kb, :],
                                    scalar1=wmin_t[:], scalar2=s15_t[:],
                                    op0=AL.subtract, op1=AL.mult)
            nc.scalar.activation(out=zsb[:, kb, 0:N], in_=t15[:],
                                 func=mybir.ActivationFunctionType.Identity,
                                 bias=gmc_c[:], scale=C2_IMM,
                                 accum_out=acc1[:, kb:kb + 1])
            inv = wtp.tile([128, N], FP32, tag="inv")
            act_recip(inv[:], t15[:], bias=GMIN, scale=C2_IMM)
            den = wtp.tile([128, N], FP32, tag="den")
            # den = (jif*RP + offs_kb) + inv  (exact Rpar + 1/G_q)
            nc.vector.affine_then_add(den[:], jif[:], inv[:], RP,
                                      offs[:, kb:kb + 1])
            act_recip(zsb[:, kb, N:2 * N], den[:], bias=0.0, scale=1.0,
                      accum=accQ[:, kb:kb + 1])

        def emit_tail(kb):
            # u_var/mv columns into zsb A-section cols N-2 / N-1
            macc = smp.tile([128, 1], FP32, tag="macc")
            nc.vector.tensor_scalar(out=macc[:], in0=acc1[:, kb:kb + 1],
                                    scalar1=1.0 / N, scalar2=CG,
                                    op0=AL.mult, op1=AL.add)
            nc.vector.tensor_scalar(out=zsb[:, kb, N - 2:N - 1], in0=macc[:],
                                    scalar1=b2_t[:], scalar2=inva_t[:],
                                    op0=AL.subtract, op1=AL.mult)
            nc.vector.tensor_scalar(out=zsb[:, kb, N - 1:N],
                                    in0=accQ[:, kb:kb + 1],
                                    scalar1=1.0 / N, scalar2=None, op0=AL.mult)

        def emit_A_kb(mt, kb):
            pa = pas[mt]
            lhsT = xt[:, mt, kb, :]
            st, sp = kb == 0, kb == KB - 1
            nc.tensor.matmul(pa[:, 0:512], lhsT, zsb[:, kb, 0:512],
                             start=st, stop=sp)
            nc.tensor.matmul(pa[:, 512:1024], lhsT, zsb[:, kb, 512:1024],
                             start=st, stop=sp)

        def emit_B_kb(mt, kb):
            pb = pbs[mt]
            lhsT = xt[:, mt, kb, :]
            st, sp = kb == 0, kb == KB - 1
            nc.tensor.matmul(pb[:, 0:512], lhsT, zsb[:, kb, N:N + 512],
                             start=st, stop=sp)
            nc.tensor.matmul(pb[:, 512:1024], lhsT, zsb[:, kb, N + 512:2 * N],
                             start=st, stop=sp)

        for kb in range(KB):
            emit_quant(kb)
            if kb + 2 < MT:
                emit_cast_tr(kb + 2)
            if kb >= 1:
                emit_tail(kb - 1)
                emit_A_kb(0, kb - 1)
                emit_A_kb(1, kb - 1)
                emit_B_kb(0, kb - 1)
                emit_B_kb(1, kb - 1)
        emit_tail(KB - 1)
        emit_A_kb(0, KB - 1)
        emit_A_kb(1, KB - 1)
        emit_B_kb(0, KB - 1)
        emit_B_kb(1, KB - 1)

        # ---------- phase 2: drain + remaining matmuls ----------
        def emit_A(mt):
            pa = pap.tile([128, N], FP32, tag="pa")
            pas[mt] = pa
            for kb in range(KB):
                emit_A_kb(mt, kb)

        def emit_B(mt):
            pb = pbp.tile([128, N], FP32, tag="pb")
            pbs[mt] = pb
            for kb in range(KB):
                emit_B_kb(mt, kb)

        def emit_post(mt):
            pa, pb = pas[mt], pbs[mt]
            nd2 = smp.tile([128, 1], FP32, tag="nd2")
            nc.vector.tensor_scalar(out=nd2[:], in0=pa[:, 1023:1024],
                                    scalar1=-1.0, scalar2=None, op0=AL.mult)
            d1s = smp.tile([128, 1], FP32, tag="d1s")
            nc.vector.tensor_scalar(out=d1s[:], in0=sumx[:, mt:mt + 1],
                                    scalar1=uc_t[:],
                                    scalar2=pa[:, 1022:1023],
                                    op0=AL.mult, op1=AL.add)
            # per-sample stats straight from PSUM (fp32)
            amax = smp.tile([128, 1], FP32, tag="amax")
            amin = smp.tile([128, 1], FP32, tag="amin")
            bmax = smp.tile([128, 1], FP32, tag="bmax")
            bmin = smp.tile([128, 1], FP32, tag="bmin")
            nc.vector.tensor_reduce(out=amax[:], in_=pa[:, 0:1022], axis=AX.X,
                                    op=AL.max)
            nc.vector.tensor_reduce(out=amin[:], in_=pa[:, 0:1022], axis=AX.X,
                                    op=AL.min)
            nc.vector.tensor_reduce(out=bmax[:], in_=pb[:], axis=AX.X,
                                    op=AL.max)
            nc.vector.tensor_reduce(out=bmin[:], in_=pb[:], axis=AX.X,
                                    op=AL.min)
            ra = smp.tile([128, 1], FP32, tag="ra")
            nc.vector.tensor_tensor(out=ra[:], in0=amax[:], in1=amin[:],
                                    op=AL.subtract)
            rbe = smp.tile([128, 1], FP32, tag="rbe")
            nc.vector.tensor_scalar(out=rbe[:], in0=bmax[:], scalar1=bmin[:],
                                    scalar2=EPS, op0=AL.subtract, op1=AL.add)
            rc = smp.tile([128, 1], FP32, tag="rc")
            nc.vector.reciprocal(rc[:], rbe[:])
            coeff = smp.tile([128, 1], FP32, tag="coeff")
            nc.vector.tensor_tensor(out=coeff[:], in0=ra[:], in1=rc[:],
                                    op=AL.mult)
            ci = smp.tile([128, 1], FP32, tag="ci")
            nc.vector.tensor_scalar(out=ci[:], in0=coeff[:], scalar1=inva_t[:],
                                    scalar2=None, op0=AL.mult)
            # bias2 = ci*nd2 + d1s  (so out = ci*B' + bias2 + bias)
            bias2 = smp.tile([128, 1], FP32, tag="bias2")
            nc.vector.tensor_scalar(out=bias2[:], in0=ci[:], scalar1=nd2[:],
                                    scalar2=d1s[:], op0=AL.mult, op1=AL.add)
            osb = obp.tile([128, N], FP32, tag="osb")
            if mt < MT - 3:
                osb1 = ob1.tile([128, N], FP32, tag="osb1")
                nc.scalar.activation(out=osb1[:], in_=pb[:],
                                     func=mybir.ActivationFunctionType.Identity,
                                     bias=bias2[:], scale=ci[:])
                nc.gpsimd.tensor_tensor(out=osb[:], in0=osb1[:], in1=biasb[:],
                                        op=AL.add)
            else:
                nc.vector.affine_then_add(osb[:], pb[:], biasb[:], ci[:],
                                          bias2[:])
            nc.sync.dma_start(out=out_d[mt * 128:(mt + 1) * 128, :], in_=osb[:])

        emit_post(0)
        emit_A(2)
        emit_B(2)
        emit_post(1)
        for mt in range(3, MT):
            emit_A(mt)
            emit_B(mt)
            emit_post(mt - 2)
        emit_post(MT - 2)
        emit_post(MT - 1)

    nc.compile()
    return nc


_NC_CACHE = None


def _get_nc():
    global _NC_CACHE
    if _NC_CACHE is None:
        _NC_CACHE = _build()
    return _NC_CACHE


def _offs_np():
    # offs[p, kb] = Rp*(K + 1 - kb*128), constant over p; the i/j dependence
    # is jif = (j - p) scaled by RP in the den affine op
    kb = np.arange(KB, dtype=np.float64)[None, :]
    p = np.zeros((128, 1), dtype=np.float64)
    return (RP * (K + 1.0 - kb * 128.0) + p).astype(np.float32)


def kernel(x, weight, bias):
    x = np.ascontiguousarray(x, np.float32)
    weight = np.ascontiguousarray(weight, np.float32)
    bias = np.ascontiguousarray(bias, np.float32)
    nc = _get_nc()
    offs = _offs_np()
    in_maps = [
        {"xs": x[c * B_SH:(c + 1) * B_SH], "w": weight, "bias": bias, "offs": offs}
        for c in range(N_CORES)
    ]
    res = run_bass_kernel_spmd(nc, in_maps, core_ids=list(range(N_CORES)))
    return np.concatenate([res.results[c]["out"] for c in range(N_CORES)], axis=0)


# revision 12
# speedup vs baseline: 1.0835x; 1.0246x over previous
"""Trainium2 Bass kernel for nn_CustomLayer_22428319220577 (v6).

Math (reference):
    G    = Gmin + (W - Wmin) * a,  a = (Gmax-Gmin)/(Wmax-Wmin)
    G_q  = round((G-Gmin)/(Gmax-Gmin)*15) * (Gmax-Gmin)/15 + Gmin
    Geff = 1/(1/G_q + Rp*((M-i)+(j+1)))
    C    = x @ Geff ;  I = x @ G_q
    coeff= (rowrange I)/(rowrange C + EPS)
    C2   = (C - rowmean C)*coeff + rowmean I
    out  = (C2 - rowsum(x)*b)/a + bias,  b = Gmin - a*Wmin

v6 formulation:
    Gc = G_q - cG ;  H = Geff
    u  = (rowmean_j G_q - b)/a, split u = uc + u_var (uc = (cG-b)/a)
    A' = x@Gc (cols 1022/1023 replaced by u_var/mv product columns)
    B' = x@H
    coeff = rowrange(A'[:, :1022]) / (rowrange(B') + EPS)
    out   = coeff/a * (B' - d2) + (d1v + uc*rowsum(x)) + bias

Schedule: all input DMA on the sync ring (x0,x1 first, then W, then x2-7);
x cast to bf16 on ACT (rowsum accumulated for the uc term) and transposed
by the DMA xbar on the scalar ring; per-kb quant pipeline (DVE t15/den,
ACT quant/inv/H) with mt0/1 A+B matmuls riding (bf16 lhsT x bf16 rhs);
drain phase with per-sample stats reduced directly from PSUM in fp32 and
the output assembled by ACT (scale/bias from PSUM) + bias add on
gpsimd/DVE.
"""
import os
import sys

sys.path.insert(0, "/opt/trn_rl_repo")

from contextlib import ExitStack

import numpy as np

import concourse.bass as bass
import concourse.tile as tile
from concourse import bacc, mybir
from concourse import bass_isa
from concourse.bass_utils import run_bass_kernel_spmd

B_FULL, K, N = 8192, 1024, 1024
N_CORES = 8
B_SH = B_FULL // N_CORES
MT = B_SH // 128
KB = K // 128

R_HRS, R_LRS, RP, BITS, EPS = 40000.0, 1000.0, 2.0, 4, 1e-8
GMIN, GMAX = 1.0 / R_HRS, 1.0 / R_LRS
GSPAN32 = np.float32(GMAX) - np.float32(GMIN)
RSPANG = float(np.float32(1.0) / GSPAN32)
LEVELS = float(2**BITS - 1)
C2_IMM = float(np.float32(GSPAN32) / np.float32(LEVELS))
CG = float(np.float32(0.5) * (np.float32(GMIN) + np.float32(GMAX)))
GMC = float(np.float32(GMIN) - np.float32(CG))

FP32 = mybir.dt.float32
F32R = mybir.dt.float32r
BF16 = mybir.dt.bfloat16
I32 = mybir.dt.int32
MM_DT = BF16


def _build():
    nc = bacc.Bacc("TRN2", target_bir_lowering=False, debug=False,
                   num_devices=N_CORES)

    xs = nc.dram_tensor("xs", [B_SH, K], FP32, kind="ExternalInput").ap()
    w = nc.dram_tensor("w", [K, N], FP32, kind="ExternalInput").ap()
    bias_d = nc.dram_tensor("bias", [N], FP32, kind="ExternalInput").ap()
    offs_d = nc.dram_tensor("offs", [128, KB], FP32, kind="ExternalInput").ap()
    out_d = nc.dram_tensor("out", [B_SH, N], FP32, kind="ExternalOutput").ap()

    AL = mybir.AluOpType
    AX = mybir.AxisListType

    def act_recip(dst, src, bias=0.0, scale=1.0, accum=None):
        # raw ACT Reciprocal: dst = 1/(src*scale + bias)
        eng = nc.scalar
        ins = [eng.lower_ap(src),
               mybir.ImmediateValue(dtype=mybir.dt.float32, value=float(bias)),
               mybir.ImmediateValue(dtype=mybir.dt.float32, value=float(scale)),
               mybir.ImmediateValue(dtype=mybir.dt.float32, value=0.0)]
        outs = [eng.lower_ap(dst)]
        if accum is not None:
            outs.append(eng.lower_ap(accum))
        eng.add_instruction(mybir.InstActivation(
            name=nc.get_next_instruction_name(),
            func=mybir.ActivationFunctionType.Reciprocal,
            ins=ins, outs=outs))

    with tile.TileContext(nc) as tc, ExitStack() as ctx:
        consts = ctx.enter_context(tc.tile_pool(name="consts", bufs=1))
        wk = ctx.enter_context(tc.tile_pool(name="wk", bufs=1))
        zp = ctx.enter_context(tc.tile_pool(name="zp", bufs=1))
        stats = ctx.enter_context(tc.tile_pool(name="stats", bufs=1))
        wtp = ctx.enter_context(tc.tile_pool(name="wtp", bufs=2))
        xin = ctx.enter_context(tc.tile_pool(name="xin", bufs=3))
        xbp = ctx.enter_context(tc.tile_pool(name="xbp", bufs=2))
        xtk = ctx.enter_context(tc.tile_pool(name="xtk", bufs=1))
        ob1 = ctx.enter_context(tc.tile_pool(name="ob1", bufs=2))
        obp = ctx.enter_context(tc.tile_pool(name="obp", bufs=2))
        smp = ctx.enter_context(tc.tile_pool(name="smp", bufs=2))
        pap = ctx.enter_context(tc.tile_pool(name="pap", bufs=2, space="PSUM"))
        pbp = ctx.enter_context(tc.tile_pool(name="pbp", bufs=2, space="PSUM"))

        # ---------- phase 0: DMAs + W stats ----------
        jif = consts.tile([128, N], FP32)
        nc.gpsimd.iota(jif[:], pattern=[[1, N]], base=0, channel_multiplier=-1,
                       allow_small_or_imprecise_dtypes=True)

        # all input DMA on the sync ring: consts, x0, x1, W0-7, x2-7 —
        # W lands by ~14us (gates phase 1), x0/1 early for the first
        # transposes, the x tail streams behind W.
        biasb = consts.tile([128, N], FP32)
        nc.sync.dma_start(
            out=biasb[:],
            in_=bass.AP(tensor=bias_d.tensor, offset=bias_d.offset,
                        ap=[[0, 128]] + bias_d.ap),
        )
        offs = consts.tile([128, KB], FP32)
        nc.sync.dma_start(out=offs[:], in_=offs_d)

        xnats = {}

        def emit_x_dma(mt):
            xnat = xin.tile([128, K], FP32, tag="xn")
            xnats[mt] = xnat
            nc.sync.dma_start(out=xnat[:], in_=xs[mt * 128:(mt + 1) * 128, :])

        emit_x_dma(0)
        emit_x_dma(1)

        w8 = wk.tile([128, KB, N], FP32)
        wmax8 = stats.tile([128, KB], FP32)
        wmin8 = stats.tile([128, KB], FP32)
        for kb in range(KB):
            nc.sync.dma_start(out=w8[:, kb, :], in_=w[kb * 128:(kb + 1) * 128, :])
            nc.vector.tensor_reduce(out=wmax8[:, kb:kb + 1], in_=w8[:, kb, :],
                                    axis=AX.X, op=AL.max)
            nc.vector.tensor_reduce(out=wmin8[:, kb:kb + 1], in_=w8[:, kb, :],
                                    axis=AX.X, op=AL.min)
        for mt in range(2, MT):
            emit_x_dma(mt)

        # x cast to bf16 on ACT (+ rowsum accum) and xbar transpose on the
        # scalar ring (issue from the ACT queue right after each cast).
        # One xt tile per mt so matmuls depend only on their own transpose.
        xts = {}
        sumx = stats.tile([128, MT], FP32)

        def emit_cast_tr(mt):
            xbf = xbp.tile([128, K], BF16, tag="xb")
            nc.scalar.activation(out=xbf[:], in_=xnats[mt][:],
                                 func=mybir.ActivationFunctionType.Identity,
                                 scale=1.0, accum_out=sumx[:, mt:mt + 1])
            xt_mt = xtk.tile([128, KB, 128], MM_DT, name=f"xt{mt}")
            xts[mt] = xt_mt
            nc.scalar.dma_start_transpose(out=xt_mt[:], in_=xbf[:])

        emit_cast_tr(0)
        emit_cast_tr(1)

        # pack [wmax | -wmin], one partition all-reduce(max)
        pk = stats.tile([128, 2], FP32)
        nc.vector.tensor_reduce(out=pk[:, 0:1], in_=wmax8[:], axis=AX.X, op=AL.max)
        wminp = stats.tile([128, 1], FP32)
        nc.vector.tensor_reduce(out=wminp[:], in_=wmin8[:], axis=AX.X, op=AL.min)
        nc.vector.tensor_scalar(out=pk[:, 1:2], in0=wminp[:], scalar1=-1.0,
                                scalar2=None, op0=AL.mult)
        pkt = stats.tile([128, 2], FP32)
        nc.gpsimd.partition_all_reduce(pkt[:], pk[:], channels=128,
                                       reduce_op=bass_isa.ReduceOp.max)

        # ---------- scalar chain ----------
        wmax_t = pkt[:, 0:1]
        wmin_t = stats.tile([128, 1], FP32)
        nc.vector.tensor_scalar(out=wmin_t[:], in0=pkt[:, 1:2], scalar1=-1.0,
                                scalar2=None, op0=AL.mult)
        span = stats.tile([128, 1], FP32)
        nc.vector.tensor_scalar(out=span[:], in0=wmax_t, scalar1=wmin_t[:],
                                scalar2=None, op0=AL.subtract)
        rspan = stats.tile([128, 1], FP32)
        nc.vector.reciprocal(rspan[:], span[:])
        s15_t = stats.tile([128, 1], FP32)
        nc.vector.tensor_scalar(out=s15_t[:], in0=rspan[:], scalar1=LEVELS,
                                scalar2=None, op0=AL.mult)
        aG_t = stats.tile([128, 1], FP32)
        nc.vector.tensor_scalar(out=aG_t[:], in0=rspan[:], scalar1=float(GSPAN32),
                                scalar2=None, op0=AL.mult)
        inva_t = stats.tile([128, 1], FP32)
        nc.vector.tensor_scalar(out=inva_t[:], in0=span[:], scalar1=RSPANG,
                                scalar2=None, op0=AL.mult)
        b_t = stats.tile([128, 1], FP32)
        nc.vector.tensor_tensor(out=b_t[:], in0=aG_t[:], in1=wmin_t[:],
                                op=AL.mult)
        nc.vector.tensor_scalar(out=b_t[:], in0=b_t[:], scalar1=-1.0,
                                scalar2=GMIN, op0=AL.mult, op1=AL.add)
        gmc_c = stats.tile([128, 1], FP32)
        nc.vector.memset(gmc_c[:], GMC)
        # u-column constant split: uc = (cG - b)/a; b2 = b + uc*a so that
        # u_var = (meanGq - b2)/a = u - uc stays small in bf16; uc*sumx is
        # re-added per-mt in the post phase.
        uc_t = stats.tile([128, 1], FP32)
        nc.vector.tensor_scalar(out=uc_t[:], in0=b_t[:], scalar1=CG,
                                scalar2=-1.0, op0=AL.subtract, op1=AL.mult)
        nc.vector.tensor_scalar(out=uc_t[:], in0=uc_t[:], scalar1=inva_t[:],
                                scalar2=None, op0=AL.mult)
        b2_t = stats.tile([128, 1], FP32)
        nc.vector.tensor_scalar(out=b2_t[:], in0=uc_t[:], scalar1=aG_t[:],
                                scalar2=b_t[:], op0=AL.mult, op1=AL.add)

        # ---------- phase 1: per-kb quant pipeline + mt0/1 matmuls ----------
        zsb = zp.tile([128, KB, 2 * N], MM_DT)
        acc1 = stats.tile([128, KB], FP32)
        accQ = stats.tile([128, KB], FP32)

        pa0 = pap.tile([128, N], FP32, tag="pa")
        pa1 = pap.tile([128, N], FP32, tag="pa")
        pb0 = pbp.tile([128, N], FP32, tag="pb")
        pb1 = pbp.tile([128, N], FP32, tag="pb")
        pas = {0: pa0, 1: pa1}
        pbs = {0: pb0, 1: pb1}

        def emit_quant(kb):
            t15 = wtp.tile([128, N], I32, tag="t15")
            nc.vector.tensor_scalar(out=t15[:], in0=w8[:, kb, :],
                                    scalar1=wmin_t[:], scalar2=s15_t[:],
                                    op0=AL.subtract, op1=AL.mult)
            nc.scalar.activation(out=zsb[:, kb, 0:N], in_=t15[:],
                                 func=mybir.ActivationFunctionType.Identity,
                                 bias=gmc_c[:], scale=C2_IMM,
                                 accum_out=acc1[:, kb:kb + 1])
            inv = wtp.tile([128, N], FP32, tag="inv")
            act_recip(inv[:], t15[:], bias=GMIN, scale=C2_IMM)
            den = wtp.tile([128, N], FP32, tag="den")
            # den = (jif*RP + offs_kb) + inv  (exact Rpar + 1/G_q)
            nc.vector.affine_then_add(den[:], jif[:], inv[:], RP,
                                      offs[:, kb:kb + 1])
            act_recip(zsb[:, kb, N:2 * N], den[:], bias=0.0, scale=1.0,
                      accum=accQ[:, kb:kb + 1])

        def emit_tail(kb):
            # u_var/mv columns into zsb A-section cols N-2 / N-1
            macc = smp.tile([128, 1], FP32, tag="macc")
            nc.vector.tensor_scalar(out=macc[:], in0=acc1[:, kb:kb + 1],
                                    scalar1=1.0 / N, scalar2=CG,
                                    op0=AL.mult, op1=AL.add)
            nc.vector.tensor_scalar(out=zsb[:, kb, N - 2:N - 1], in0=macc[:],
                                    scalar1=b2_t[:], scalar2=inva_t[:],
                                    op0=AL.subtract, op1=AL.mult)
            nc.vector.tensor_scalar(out=zsb[:, kb, N - 1:N],
                                    in0=accQ[:, kb:kb + 1],
                                    scalar1=1.0 / N, scalar2=None, op0=AL.mult)

        def emit_A_kb(mt, kb):
            pa = pas[mt]
            lhsT = xts[mt][:, kb, :]
            st, sp = kb == 0, kb == KB - 1
            nc.tensor.matmul(pa[:, 0:512], lhsT, zsb[:, kb, 0:512],
                             start=st, stop=sp)
            nc.tensor.matmul(pa[:, 512:1024], lhsT, zsb[:, kb, 512:1024],
                             start=st, stop=sp)

        def emit_B_kb(mt, kb):
            pb = pbs[mt]
            lhsT = xts[mt][:, kb, :]
            st, sp = kb == 0, kb == KB - 1
            nc.tensor.matmul(pb[:, 0:512], lhsT, zsb[:, kb, N:N + 512],
                             start=st, stop=sp)
            nc.tensor.matmul(pb[:, 512:1024], lhsT, zsb[:, kb, N + 512:2 * N],
                             start=st, stop=sp)

        for kb in range(KB):
            emit_quant(kb)
            if kb + 2 < MT:
                emit_cast_tr(kb + 2)
            if kb >= 1:
                emit_tail(kb - 1)
                emit_A_kb(0, kb - 1)
                emit_A_kb(1, kb - 1)
                emit_B_kb(0, kb - 1)
                emit_B_kb(1, kb - 1)
        emit_tail(KB - 1)
        emit_A_kb(0, KB - 1)
        emit_A_kb(1, KB - 1)
        emit_B_kb(0, KB - 1)
        emit_B_kb(1, KB - 1)

        # ---------- phase 2: drain + remaining matmuls ----------
        def emit_A(mt):
            pa = pap.tile([128, N], FP32, tag="pa")
            pas[mt] = pa
            for kb in range(KB):
                emit_A_kb(mt, kb)

        def emit_B(mt):
            pb = pbp.tile([128, N], FP32, tag="pb")
            pbs[mt] = pb
            for kb in range(KB):
                emit_B_kb(mt, kb)

        def emit_post(mt):
            pa, pb = pas[mt], pbs[mt]
            nd2 = smp.tile([128, 1], FP32, tag="nd2")
            nc.vector.tensor_scalar(out=nd2[:], in0=pa[:, 1023:1024],
                                    scalar1=-1.0, scalar2=None, op0=AL.mult)
            d1s = smp.tile([128, 1], FP32, tag="d1s")
            nc.vector.tensor_scalar(out=d1s[:], in0=sumx[:, mt:mt + 1],
                                    scalar1=uc_t[:],
                                    scalar2=pa[:, 1022:1023],
                                    op0=AL.mult, op1=AL.add)
            # per-sample stats straight from PSUM (fp32)
            amax = smp.tile([128, 1], FP32, tag="amax")
            amin = smp.tile([128, 1], FP32, tag="amin")
            bmax = smp.tile([128, 1], FP32, tag="bmax")
            bmin = smp.tile([128, 1], FP32, tag="bmin")
            nc.vector.tensor_reduce(out=amax[:], in_=pa[:, 0:1022], axis=AX.X,
                                    op=AL.max)
            nc.vector.tensor_reduce(out=amin[:], in_=pa[:, 0:1022], axis=AX.X,
                                    op=AL.min)
            nc.vector.tensor_reduce(out=bmax[:], in_=pb[:], axis=AX.X,
                                    op=AL.max)
            nc.vector.tensor_reduce(out=bmin[:], in_=pb[:], axis=AX.X,
                                    op=AL.min)
            ra = smp.tile([128, 1], FP32, tag="ra")
            nc.vector.tensor_tensor(out=ra[:], in0=amax[:], in1=amin[:],
                                    op=AL.subtract)
            rbe = smp.tile([128, 1], FP32, tag="rbe")
            nc.vector.tensor_scalar(out=rbe[:], in0=bmax[:], scalar1=bmin[:],
                                    scalar2=EPS, op0=AL.subtract, op1=AL.add)
            rc = smp.tile([128, 1], FP32, tag="rc")
            nc.vector.reciprocal(rc[:], rbe[:])
            coeff = smp.tile([128, 1], FP32, tag="coeff")
            nc.vector.tensor_tensor(out=coeff[:], in0=ra[:], in1=rc[:],
                                    op=AL.mult)
            ci = smp.tile([128, 1], FP32, tag="ci")
            nc.vector.tensor_scalar(out=ci[:], in0=coeff[:], scalar1=inva_t[:],
                                    scalar2=None, op0=AL.mult)
            # bias2 = ci*nd2 + d1s  (so out = ci*B' + bias2 + bias)
            bias2 = smp.tile([128, 1], FP32, tag="bias2")
            nc.vector.tensor_scalar(out=bias2[:], in0=ci[:], scalar1=nd2[:],
                                    scalar2=d1s[:], op0=AL.mult, op1=AL.add)
            osb = obp.tile([128, N], FP32, tag="osb")
            if mt < MT - 3:
                osb1 = ob1.tile([128, N], FP32, tag="osb1")
                nc.scalar.activation(out=osb1[:], in_=pb[:],
                                     func=mybir.ActivationFunctionType.Identity,
                                     bias=bias2[:], scale=ci[:])
                nc.gpsimd.tensor_tensor(out=osb[:], in0=osb1[:], in1=biasb[:],
                                        op=AL.add)
            else:
                nc.vector.affine_then_add(osb[:], pb[:], biasb[:], ci[:],
                                          bias2[:])
            nc.sync.dma_start(out=out_d[mt * 128:(mt + 1) * 128, :], in_=osb[:])

        emit_post(0)
        emit_A(2)
        emit_B(2)
        emit_post(1)
        for mt in range(3, MT):
            emit_A(mt)
            emit_B(mt)
            emit_post(mt - 2)
        emit_post(MT - 2)
        emit_post(MT - 1)

    nc.compile()
    return nc


_NC_CACHE = None


def _get_nc():
    global _NC_CACHE
    if _NC_CACHE is None:
        _NC_CACHE = _build()
    return _NC_CACHE


def _offs_np():
    # offs[p, kb] = Rp*(K + 1 - kb*128), constant over p; the i/j dependence
    # is jif = (j - p) scaled by RP in the den affine op
    kb = np.arange(KB, dtype=np.float64)[None, :]
    p = np.zeros((128, 1), dtype=np.float64)
    return (RP * (K + 1.0 - kb * 128.0) + p).astype(np.float32)


def kernel(x, weight, bias):
    x = np.ascontiguousarray(x, np.float32)
    weight = np.ascontiguousarray(weight, np.float32)
    bias = np.ascontiguousarray(bias, np.float32)
    nc = _get_nc()
    offs = _offs_np()
    in_maps = [
        {"xs": x[c * B_SH:(c + 1) * B_SH], "w": weight, "bias": bias, "offs": offs}
        for c in range(N_CORES)
    ]
    res = run_bass_kernel_spmd(nc, in_maps, core_ids=list(range(N_CORES)))
    return np.concatenate([res.results[c]["out"] for c in range(N_CORES)], axis=0)


# revision 13
# speedup vs baseline: 1.1041x; 1.0190x over previous
"""Trainium2 Bass kernel for nn_CustomLayer_22428319220577 (v6).

Math (reference):
    G    = Gmin + (W - Wmin) * a,  a = (Gmax-Gmin)/(Wmax-Wmin)
    G_q  = round((G-Gmin)/(Gmax-Gmin)*15) * (Gmax-Gmin)/15 + Gmin
    Geff = 1/(1/G_q + Rp*((M-i)+(j+1)))
    C    = x @ Geff ;  I = x @ G_q
    coeff= (rowrange I)/(rowrange C + EPS)
    C2   = (C - rowmean C)*coeff + rowmean I
    out  = (C2 - rowsum(x)*b)/a + bias,  b = Gmin - a*Wmin

v6 formulation:
    Gc = G_q - cG ;  H = Geff
    u  = (rowmean_j G_q - b)/a, split u = uc + u_var (uc = (cG-b)/a)
    A' = x@Gc (cols 1022/1023 replaced by u_var/mv product columns)
    B' = x@H
    coeff = rowrange(A'[:, :1022]) / (rowrange(B') + EPS)
    out   = coeff/a * (B' - d2) + (d1v + uc*rowsum(x)) + bias

Schedule: all input DMA on the sync ring (x0,x1 first, then W, then x2-7);
x cast to bf16 on ACT (rowsum accumulated for the uc term) and transposed
by the DMA xbar on the scalar ring; per-kb quant pipeline (DVE t15/den,
ACT quant/inv/H) with mt0/1 A+B matmuls riding (bf16 lhsT x bf16 rhs);
drain phase with per-sample stats reduced directly from PSUM in fp32 and
the output assembled by ACT (scale/bias from PSUM) + bias add on
gpsimd/DVE.
"""
import os
import sys

sys.path.insert(0, "/opt/trn_rl_repo")

from contextlib import ExitStack

import numpy as np

import concourse.bass as bass
import concourse.tile as tile
from concourse import bacc, mybir
from concourse import bass_isa
from concourse.bass_utils import run_bass_kernel_spmd

B_FULL, K, N = 8192, 1024, 1024
N_CORES = 8
B_SH = B_FULL // N_CORES
MT = B_SH // 128
KB = K // 128

R_HRS, R_LRS, RP, BITS, EPS = 40000.0, 1000.0, 2.0, 4, 1e-8
GMIN, GMAX = 1.0 / R_HRS, 1.0 / R_LRS
GSPAN32 = np.float32(GMAX) - np.float32(GMIN)
RSPANG = float(np.float32(1.0) / GSPAN32)
LEVELS = float(2**BITS - 1)
C2_IMM = float(np.float32(GSPAN32) / np.float32(LEVELS))
CG = float(np.float32(0.5) * (np.float32(GMIN) + np.float32(GMAX)))
GMC = float(np.float32(GMIN) - np.float32(CG))

FP32 = mybir.dt.float32
F32R = mybir.dt.float32r
BF16 = mybir.dt.bfloat16
I32 = mybir.dt.int32
MM_DT = BF16


def _build():
    nc = bacc.Bacc("TRN2", target_bir_lowering=False, debug=False,
                   num_devices=N_CORES)

    xs = nc.dram_tensor("xs", [B_SH, K], FP32, kind="ExternalInput").ap()
    w = nc.dram_tensor("w", [K, N], FP32, kind="ExternalInput").ap()
    bias_d = nc.dram_tensor("bias", [N], FP32, kind="ExternalInput").ap()
    offs_d = nc.dram_tensor("offs", [128, KB], FP32, kind="ExternalInput").ap()
    out_d = nc.dram_tensor("out", [B_SH, N], FP32, kind="ExternalOutput").ap()

    AL = mybir.AluOpType
    AX = mybir.AxisListType

    def act_recip(dst, src, bias=0.0, scale=1.0, accum=None):
        # raw ACT Reciprocal: dst = 1/(src*scale + bias)
        eng = nc.scalar
        ins = [eng.lower_ap(src),
               mybir.ImmediateValue(dtype=mybir.dt.float32, value=float(bias)),
               mybir.ImmediateValue(dtype=mybir.dt.float32, value=float(scale)),
               mybir.ImmediateValue(dtype=mybir.dt.float32, value=0.0)]
        outs = [eng.lower_ap(dst)]
        if accum is not None:
            outs.append(eng.lower_ap(accum))
        eng.add_instruction(mybir.InstActivation(
            name=nc.get_next_instruction_name(),
            func=mybir.ActivationFunctionType.Reciprocal,
            ins=ins, outs=outs))

    with tile.TileContext(nc) as tc, ExitStack() as ctx:
        consts = ctx.enter_context(tc.tile_pool(name="consts", bufs=1))
        wk = ctx.enter_context(tc.tile_pool(name="wk", bufs=1))
        zp = ctx.enter_context(tc.tile_pool(name="zp", bufs=1))
        stats = ctx.enter_context(tc.tile_pool(name="stats", bufs=1))
        wtp = ctx.enter_context(tc.tile_pool(name="wtp", bufs=2))
        xin = ctx.enter_context(tc.tile_pool(name="xin", bufs=3))
        xbp = ctx.enter_context(tc.tile_pool(name="xbp", bufs=2))
        xtk = ctx.enter_context(tc.tile_pool(name="xtk", bufs=1))
        ob1 = ctx.enter_context(tc.tile_pool(name="ob1", bufs=2))
        obp = ctx.enter_context(tc.tile_pool(name="obp", bufs=2))
        smp = ctx.enter_context(tc.tile_pool(name="smp", bufs=2))
        pap = ctx.enter_context(tc.tile_pool(name="pap", bufs=2, space="PSUM"))
        pbp = ctx.enter_context(tc.tile_pool(name="pbp", bufs=2, space="PSUM"))

        # ---------- phase 0: DMAs + W stats ----------
        jif = consts.tile([128, N], FP32)
        nc.gpsimd.iota(jif[:], pattern=[[1, N]], base=0, channel_multiplier=-1,
                       allow_small_or_imprecise_dtypes=True)

        # input DMA split across both HWDGE rings (sync + scalar) so W
        # lands in ~12us: sync ring: x0, W-even, x-even; scalar ring:
        # offs, x1, W-odd, then (issued later from the ACT queue) the
        # first transposes, x-odd tail and biasb.
        offs = consts.tile([128, KB], FP32)
        nc.scalar.dma_start(out=offs[:], in_=offs_d)

        xnats = {}

        def emit_x_dma(mt, eng):
            xnat = xin.tile([128, K], FP32, tag="xn")
            xnats[mt] = xnat
            eng.dma_start(out=xnat[:], in_=xs[mt * 128:(mt + 1) * 128, :])

        emit_x_dma(0, nc.sync)
        emit_x_dma(1, nc.scalar)

        w8 = wk.tile([128, KB, N], FP32)
        wmax8 = stats.tile([128, KB], FP32)
        wmin8 = stats.tile([128, KB], FP32)
        for kb in range(KB):
            dq = nc.sync if kb % 2 == 0 else nc.scalar
            dq.dma_start(out=w8[:, kb, :], in_=w[kb * 128:(kb + 1) * 128, :])
            nc.vector.tensor_reduce(out=wmax8[:, kb:kb + 1], in_=w8[:, kb, :],
                                    axis=AX.X, op=AL.max)
            nc.vector.tensor_reduce(out=wmin8[:, kb:kb + 1], in_=w8[:, kb, :],
                                    axis=AX.X, op=AL.min)
        for mt in (2, 4, 6):
            emit_x_dma(mt, nc.sync)

        # x cast to bf16 on ACT (+ rowsum accum) and xbar transpose on the
        # scalar ring (issue from the ACT queue right after each cast).
        # One xt tile per mt so matmuls depend only on their own transpose.
        xts = {}
        sumx = stats.tile([128, MT], FP32)

        def emit_cast_tr(mt):
            xbf = xbp.tile([128, K], BF16, tag="xb")
            nc.scalar.activation(out=xbf[:], in_=xnats[mt][:],
                                 func=mybir.ActivationFunctionType.Identity,
                                 scale=1.0, accum_out=sumx[:, mt:mt + 1])
            xt_mt = xtk.tile([128, KB, 128], MM_DT, name=f"xt{mt}")
            xts[mt] = xt_mt
            nc.scalar.dma_start_transpose(out=xt_mt[:], in_=xbf[:])

        emit_cast_tr(0)
        emit_cast_tr(1)
        for mt in (3, 5, 7):
            emit_x_dma(mt, nc.scalar)
        biasb = consts.tile([128, N], FP32)
        nc.scalar.dma_start(
            out=biasb[:],
            in_=bass.AP(tensor=bias_d.tensor, offset=bias_d.offset,
                        ap=[[0, 128]] + bias_d.ap),
        )

        # pack [wmax | -wmin], one partition all-reduce(max)
        pk = stats.tile([128, 2], FP32)
        nc.vector.tensor_reduce(out=pk[:, 0:1], in_=wmax8[:], axis=AX.X, op=AL.max)
        wminp = stats.tile([128, 1], FP32)
        nc.vector.tensor_reduce(out=wminp[:], in_=wmin8[:], axis=AX.X, op=AL.min)
        nc.vector.tensor_scalar(out=pk[:, 1:2], in0=wminp[:], scalar1=-1.0,
                                scalar2=None, op0=AL.mult)
        pkt = stats.tile([128, 2], FP32)
        nc.gpsimd.partition_all_reduce(pkt[:], pk[:], channels=128,
                                       reduce_op=bass_isa.ReduceOp.max)

        # ---------- scalar chain ----------
        wmax_t = pkt[:, 0:1]
        wmin_t = stats.tile([128, 1], FP32)
        nc.vector.tensor_scalar(out=wmin_t[:], in0=pkt[:, 1:2], scalar1=-1.0,
                                scalar2=None, op0=AL.mult)
        span = stats.tile([128, 1], FP32)
        nc.vector.tensor_scalar(out=span[:], in0=wmax_t, scalar1=wmin_t[:],
                                scalar2=None, op0=AL.subtract)
        rspan = stats.tile([128, 1], FP32)
        nc.vector.reciprocal(rspan[:], span[:])
        s15_t = stats.tile([128, 1], FP32)
        nc.vector.tensor_scalar(out=s15_t[:], in0=rspan[:], scalar1=LEVELS,
                                scalar2=None, op0=AL.mult)
        aG_t = stats.tile([128, 1], FP32)
        nc.vector.tensor_scalar(out=aG_t[:], in0=rspan[:], scalar1=float(GSPAN32),
                                scalar2=None, op0=AL.mult)
        inva_t = stats.tile([128, 1], FP32)
        nc.vector.tensor_scalar(out=inva_t[:], in0=span[:], scalar1=RSPANG,
                                scalar2=None, op0=AL.mult)
        b_t = stats.tile([128, 1], FP32)
        nc.vector.tensor_tensor(out=b_t[:], in0=aG_t[:], in1=wmin_t[:],
                                op=AL.mult)
        nc.vector.tensor_scalar(out=b_t[:], in0=b_t[:], scalar1=-1.0,
                                scalar2=GMIN, op0=AL.mult, op1=AL.add)
        gmc_c = stats.tile([128, 1], FP32)
        nc.vector.memset(gmc_c[:], GMC)
        # u-column constant split: uc = (cG - b)/a; b2 = b + uc*a so that
        # u_var = (meanGq - b2)/a = u - uc stays small in bf16; uc*sumx is
        # re-added per-mt in the post phase.
        uc_t = stats.tile([128, 1], FP32)
        nc.vector.tensor_scalar(out=uc_t[:], in0=b_t[:], scalar1=CG,
                                scalar2=-1.0, op0=AL.subtract, op1=AL.mult)
        nc.vector.tensor_scalar(out=uc_t[:], in0=uc_t[:], scalar1=inva_t[:],
                                scalar2=None, op0=AL.mult)
        b2_t = stats.tile([128, 1], FP32)
        nc.vector.tensor_scalar(out=b2_t[:], in0=uc_t[:], scalar1=aG_t[:],
                                scalar2=b_t[:], op0=AL.mult, op1=AL.add)

        # ---------- phase 1: per-kb quant pipeline + mt0/1 matmuls ----------
        zsb = zp.tile([128, KB, 2 * N], MM_DT)
        acc1 = stats.tile([128, KB], FP32)
        accQ = stats.tile([128, KB], FP32)

        pa0 = pap.tile([128, N], FP32, tag="pa")
        pa1 = pap.tile([128, N], FP32, tag="pa")
        pb0 = pbp.tile([128, N], FP32, tag="pb")
        pb1 = pbp.tile([128, N], FP32, tag="pb")
        pas = {0: pa0, 1: pa1}
        pbs = {0: pb0, 1: pb1}

        def emit_quant(kb):
            t15 = wtp.tile([128, N], I32, tag="t15")
            nc.vector.tensor_scalar(out=t15[:], in0=w8[:, kb, :],
                                    scalar1=wmin_t[:], scalar2=s15_t[:],
                                    op0=AL.subtract, op1=AL.mult)
            nc.scalar.activation(out=zsb[:, kb, 0:N], in_=t15[:],
                                 func=mybir.ActivationFunctionType.Identity,
                                 bias=gmc_c[:], scale=C2_IMM,
                                 accum_out=acc1[:, kb:kb + 1])
            inv = wtp.tile([128, N], FP32, tag="inv")
            act_recip(inv[:], t15[:], bias=GMIN, scale=C2_IMM)
            den = wtp.tile([128, N], FP32, tag="den")
            # den = (jif*RP + offs_kb) + inv  (exact Rpar + 1/G_q)
            nc.vector.affine_then_add(den[:], jif[:], inv[:], RP,
                                      offs[:, kb:kb + 1])
            act_recip(zsb[:, kb, N:2 * N], den[:], bias=0.0, scale=1.0,
                      accum=accQ[:, kb:kb + 1])

        def emit_tail(kb):
            # u_var/mv columns into zsb A-section cols N-2 / N-1
            macc = smp.tile([128, 1], FP32, tag="macc")
            nc.vector.tensor_scalar(out=macc[:], in0=acc1[:, kb:kb + 1],
                                    scalar1=1.0 / N, scalar2=CG,
                                    op0=AL.mult, op1=AL.add)
            nc.vector.tensor_scalar(out=zsb[:, kb, N - 2:N - 1], in0=macc[:],
                                    scalar1=b2_t[:], scalar2=inva_t[:],
                                    op0=AL.subtract, op1=AL.mult)
            nc.vector.tensor_scalar(out=zsb[:, kb, N - 1:N],
                                    in0=accQ[:, kb:kb + 1],
                                    scalar1=1.0 / N, scalar2=None, op0=AL.mult)

        def emit_A_kb(mt, kb):
            pa = pas[mt]
            lhsT = xts[mt][:, kb, :]
            st, sp = kb == 0, kb == KB - 1
            nc.tensor.matmul(pa[:, 0:512], lhsT, zsb[:, kb, 0:512],
                             start=st, stop=sp)
            nc.tensor.matmul(pa[:, 512:1024], lhsT, zsb[:, kb, 512:1024],
                             start=st, stop=sp)

        def emit_B_kb(mt, kb):
            pb = pbs[mt]
            lhsT = xts[mt][:, kb, :]
            st, sp = kb == 0, kb == KB - 1
            nc.tensor.matmul(pb[:, 0:512], lhsT, zsb[:, kb, N:N + 512],
                             start=st, stop=sp)
            nc.tensor.matmul(pb[:, 512:1024], lhsT, zsb[:, kb, N + 512:2 * N],
                             start=st, stop=sp)

        for kb in range(KB):
            emit_quant(kb)
            if kb + 2 < MT:
                emit_cast_tr(kb + 2)
            if kb >= 1:
                emit_tail(kb - 1)
                emit_A_kb(0, kb - 1)
                emit_A_kb(1, kb - 1)
                emit_B_kb(0, kb - 1)
                emit_B_kb(1, kb - 1)
        emit_tail(KB - 1)
        emit_A_kb(0, KB - 1)
        emit_A_kb(1, KB - 1)
        emit_B_kb(0, KB - 1)
        emit_B_kb(1, KB - 1)

        # ---------- phase 2: drain + remaining matmuls ----------
        def emit_A(mt):
            pa = pap.tile([128, N], FP32, tag="pa")
            pas[mt] = pa
            for kb in range(KB):
                emit_A_kb(mt, kb)

        def emit_B(mt):
            pb = pbp.tile([128, N], FP32, tag="pb")
            pbs[mt] = pb
            for kb in range(KB):
                emit_B_kb(mt, kb)

        def emit_post(mt):
            pa, pb = pas[mt], pbs[mt]
            nd2 = smp.tile([128, 1], FP32, tag="nd2")
            nc.vector.tensor_scalar(out=nd2[:], in0=pa[:, 1023:1024],
                                    scalar1=-1.0, scalar2=None, op0=AL.mult)
            d1s = smp.tile([128, 1], FP32, tag="d1s")
            nc.vector.tensor_scalar(out=d1s[:], in0=sumx[:, mt:mt + 1],
                                    scalar1=uc_t[:],
                                    scalar2=pa[:, 1022:1023],
                                    op0=AL.mult, op1=AL.add)
            # per-sample stats straight from PSUM (fp32)
            amax = smp.tile([128, 1], FP32, tag="amax")
            amin = smp.tile([128, 1], FP32, tag="amin")
            bmax = smp.tile([128, 1], FP32, tag="bmax")
            bmin = smp.tile([128, 1], FP32, tag="bmin")
            nc.vector.tensor_reduce(out=amax[:], in_=pa[:, 0:1022], axis=AX.X,
                                    op=AL.max)
            nc.vector.tensor_reduce(out=amin[:], in_=pa[:, 0:1022], axis=AX.X,
                                    op=AL.min)
            nc.vector.tensor_reduce(out=bmax[:], in_=pb[:], axis=AX.X,
                                    op=AL.max)
            nc.vector.tensor_reduce(out=bmin[:], in_=pb[:], axis=AX.X,
                                    op=AL.min)
            ra = smp.tile([128, 1], FP32, tag="ra")
            nc.vector.tensor_tensor(out=ra[:], in0=amax[:], in1=amin[:],
                                    op=AL.subtract)
            rbe = smp.tile([128, 1], FP32, tag="rbe")
            nc.vector.tensor_scalar(out=rbe[:], in0=bmax[:], scalar1=bmin[:],
                                    scalar2=EPS, op0=AL.subtract, op1=AL.add)
            rc = smp.tile([128, 1], FP32, tag="rc")
            nc.vector.reciprocal(rc[:], rbe[:])
            coeff = smp.tile([128, 1], FP32, tag="coeff")
            nc.vector.tensor_tensor(out=coeff[:], in0=ra[:], in1=rc[:],
                                    op=AL.mult)
            ci = smp.tile([128, 1], FP32, tag="ci")
            nc.vector.tensor_scalar(out=ci[:], in0=coeff[:], scalar1=inva_t[:],
                                    scalar2=None, op0=AL.mult)
            # bias2 = ci*nd2 + d1s  (so out = ci*B' + bias2 + bias)
            bias2 = smp.tile([128, 1], FP32, tag="bias2")
            nc.vector.tensor_scalar(out=bias2[:], in0=ci[:], scalar1=nd2[:],
                                    scalar2=d1s[:], op0=AL.mult, op1=AL.add)
            osb = obp.tile([128, N], FP32, tag="osb")
            if mt < MT - 3:
                osb1 = ob1.tile([128, N], FP32, tag="osb1")
                nc.scalar.activation(out=osb1[:], in_=pb[:],
                                     func=mybir.ActivationFunctionType.Identity,
                                     bias=bias2[:], scale=ci[:])
                nc.gpsimd.tensor_tensor(out=osb[:], in0=osb1[:], in1=biasb[:],
                                        op=AL.add)
            else:
                nc.vector.affine_then_add(osb[:], pb[:], biasb[:], ci[:],
                                          bias2[:])
            nc.sync.dma_start(out=out_d[mt * 128:(mt + 1) * 128, :], in_=osb[:])

        emit_post(0)
        emit_A(2)
        emit_B(2)
        emit_post(1)
        for mt in range(3, MT):
            emit_A(mt)
            emit_B(mt)
            emit_post(mt - 2)
        emit_post(MT - 2)
        emit_post(MT - 1)

    nc.compile()
    return nc


_NC_CACHE = None


def _get_nc():
    global _NC_CACHE
    if _NC_CACHE is None:
        _NC_CACHE = _build()
    return _NC_CACHE


def _offs_np():
    # offs[p, kb] = Rp*(K + 1 - kb*128), constant over p; the i/j dependence
    # is jif = (j - p) scaled by RP in the den affine op
    kb = np.arange(KB, dtype=np.float64)[None, :]
    p = np.zeros((128, 1), dtype=np.float64)
    return (RP * (K + 1.0 - kb * 128.0) + p).astype(np.float32)


def kernel(x, weight, bias):
    x = np.ascontiguousarray(x, np.float32)
    weight = np.ascontiguousarray(weight, np.float32)
    bias = np.ascontiguousarray(bias, np.float32)
    nc = _get_nc()
    offs = _offs_np()
    in_maps = [
        {"xs": x[c * B_SH:(c + 1) * B_SH], "w": weight, "bias": bias, "offs": offs}
        for c in range(N_CORES)
    ]
    res = run_bass_kernel_spmd(nc, in_maps, core_ids=list(range(N_CORES)))
    return np.concatenate([res.results[c]["out"] for c in range(N_CORES)], axis=0)
